# revision 11
# baseline (speedup 1.0000x reference)
"""TRN2 Bass kernel for nn_Attention_70257075028315.

reference:
    scores = einsum('bqd,bkd->bqk', query, key)       # B=8, Nq=Nk=2048, D=512
    probs  = softmax(scores, -1)
    out    = einsum('bqk,bkd->bqd', probs, key)

Sharding: batch b -> NeuronCore b (data parallel, fully local attention).

Per-core program (q/k: [2048, 512] fp32):
  Phase A/B: load K and Q in [128, 512] tiles; PE-transpose each into
    kT/qT [128(d), 4(dc), 16(kk/q tile), 128] stored as float32r (rounded by
    the PSUM->SBUF copy); K also cast to float32r natural layout for PV.
  Phase C (per q-tile, software-pipelined):
    S    = qT.T @ kT           accumulated over 4 d-chunks -> PSUM [128, 4, 512]
    max  per 512-chunk on DVE, combined and negated -> bias
    p    = exp(S - max) via one ACT pass PSUM->SBUF (f32r), fused row-sum
    pT   = PE-transpose of p (16x [128,128]) -> PSUM -> ACT copy to SBUF f32r
    o    = pT.T @ k_pv         accumulated over 16 kk-tiles -> PSUM [128, 512]
    out  = o * (1/rowsum)      on DVE, then DMA to DRAM.

float32r matmuls run at 1 cyc/row (vs 4 for fp32) with ~1.5e-2 max abs error
on N(0,512) scores (measured on HW) -> ~1e-3 relative error on the output.
"""

import numpy as np

import concourse.bass as bass
import concourse.tile as tile
import concourse.mybir as mybir
from concourse import bacc
from concourse.bass_utils import run_bass_kernel_spmd
from concourse.masks import make_identity

FP32 = mybir.dt.float32
FP32R = mybir.dt.float32r
AF = mybir.ActivationFunctionType

B, NQ, NK, D = 8, 2048, 2048, 512
P = 128
NKT = NK // P   # 16 kk tiles
NQT = NQ // P   # 16 q tiles
NDC = D // P    # 4 d chunks
NCH = NK // 512  # 4 score chunks of 512


def build(score_dtype=FP32R, repeat_c=1):
    nc = bacc.Bacc("TRN2", target_bir_lowering=False, debug=False)
    q_d = nc.dram_tensor("query", [NQ, D], FP32, kind="ExternalInput").ap()
    k_d = nc.dram_tensor("key", [NK, D], FP32, kind="ExternalInput").ap()
    out_d = nc.dram_tensor("out", [NQ, D], FP32, kind="ExternalOutput").ap()

    q_tiles_d = q_d.rearrange("(t p) d -> t p d", p=P)
    k_tiles_d = k_d.rearrange("(t p) d -> t p d", p=P)
    out_tiles_d = out_d.rearrange("(t p) d -> t p d", p=P)

    with tile.TileContext(nc) as tc:
        _body(tc, q_tiles_d, k_tiles_d, out_tiles_d, score_dtype, repeat_c)
    nc.compile()
    return nc


def _body(tc, q_tiles_d, k_tiles_d, out_tiles_d, score_dtype, repeat_c):
    from contextlib import ExitStack

    nc = tc.nc
    with ExitStack() as ctx:
        persist = ctx.enter_context(tc.tile_pool(name="persist", bufs=1))
        work = ctx.enter_context(tc.tile_pool(name="work", bufs=2))
        small = ctx.enter_context(tc.tile_pool(name="small", bufs=3))
        ps_s = ctx.enter_context(tc.tile_pool(name="ps_s", bufs=4, space="PSUM"))
        ps_tr = ctx.enter_context(tc.tile_pool(name="ps_tr", bufs=2, space="PSUM"))
        ps_pv = ctx.enter_context(tc.tile_pool(name="ps_pv", bufs=2, space="PSUM"))

        ident = persist.tile([P, P], FP32)
        make_identity(nc, ident[:])
        ident_r = persist.tile([P, P], FP32R)
        nc.vector.tensor_copy(ident_r[:], ident[:])

        # Transposed operands: [d%128, d-chunk, kk-tile, 128]
        kT = persist.tile([P, NDC, NKT, P], score_dtype)
        qT = persist.tile([P, NDC, NQT, P], score_dtype)
        k_pv = persist.tile([P, NKT, 512 // P, P], FP32R)  # natural [kk, d]

        # ---- Phase A/B: load, round to f32r, transpose ----
        # All PSUM->SBUF copies ride the DMA engines (f32r -> f32r, already
        # rounded); compute engines only do the f32 -> f32r rounding casts.
        with tc.tile_pool(name="load", bufs=6) as load:
            for src_d, dstT, pv in ((k_tiles_d, kT, k_pv), (q_tiles_d, qT, None)):
                for g in range(4):  # groups of 4 tiles
                    rtiles = []
                    for j in range(4):
                        t = load.tile([P, D], FP32, tag="ld")
                        nc.sync.dma_start(t[:], src_d[g * 4 + j])
                        if pv is not None:
                            r = pv[:, g * 4 + j]
                            nc.vector.tensor_copy(
                                r, t[:].rearrange("p (a b) -> p a b", b=P)
                            )
                            rtiles.append(pv[:, g * 4 + j].rearrange("p a b -> p (a b)"))
                        else:
                            r = load.tile([P, D], FP32R, tag="ldr")
                            nc.vector.tensor_copy(r[:], t[:])
                            rtiles.append(r[:])
                    for dc in range(NDC):
                        ptr = ps_tr.tile([P, 4, P], FP32R, tag="tr")
                        for j in range(4):
                            nc.tensor.transpose(
                                ptr[:, j, :],
                                rtiles[j][:, dc * P : (dc + 1) * P],
                                ident_r[:],
                            )
                        nc.scalar.copy(dstT[:, dc, g * 4 : (g + 1) * 4, :], ptr[:])

        # ---- Phase C: attention over q tiles, software-pipelined ----
        def emit_S(i):
            """S matmuls (4 separate PSUM chunk tiles) + chunk maxes + negmax."""
            chunks = []
            m4 = small.tile([P, NCH], FP32, tag="m4")
            negmax = small.tile([P, 1], FP32, tag="negmax")
            last_mm = None
            for c in range(NCH):
                psc = ps_s.tile([P, 512], FP32, tag="s")
                for dc in range(NDC):
                    last_mm = nc.tensor.matmul(
                        psc[:],
                        lhsT=qT[:, dc, i, :],
                        rhs=kT[:, dc, c * 4 : (c + 1) * 4, :],
                        start=(dc == 0),
                        stop=(dc == NDC - 1),
                    )
                nc.vector.reduce_max(
                    m4[:, c : c + 1], psc[:], axis=mybir.AxisListType.X
                )
                chunks.append(psc)
            nc.vector.reduce_max(
                negmax[:], m4[:], axis=mybir.AxisListType.X, negate=True
            )
            return chunks, negmax, last_mm

        def emit_E(i, chunks, negmax):
            """exp(S - max) per chunk -> p (f32r) + partial row-sums; 1/sum."""
            p = work.tile([P, NCH, 512], FP32R, tag="p")
            rs4 = small.tile([P, NCH], FP32, tag="rs4")
            rowsum = small.tile([P, 1], FP32, tag="rowsum")
            rinv = small.tile([P, 1], FP32, tag="rinv")
            for c in range(NCH):
                nc.scalar.activation(
                    p[:, c, :], chunks[c][:], AF.Exp, bias=negmax[:],
                    accum_out=rs4[:, c : c + 1],
                )
            nc.vector.reduce_sum(rowsum[:], rs4[:], axis=mybir.AxisListType.X)
            nc.vector.reciprocal(rinv[:], rowsum[:])
            return p, rinv

        def emit_T(i, p):
            """Transpose p -> pT [128(kk), 16 tiles, 128(q)] f32r."""
            pT = work.tile([P, NKT, P], FP32R, tag="pT")
            for g in range(4):
                ptr = ps_tr.tile([P, 4, P], FP32R, tag="tr")
                for j in range(4):
                    nc.tensor.transpose(
                        ptr[:, j, :],
                        p[:, g, j * P : (j + 1) * P],
                        ident_r[:],
                    )
                eng = nc.scalar.copy if g % 2 == 0 else nc.vector.tensor_copy
                eng(pT[:, g * 4 : (g + 1) * 4, :], ptr[:])
            return pT

        def emit_PV(i, pT, rinv, after=None):
            psum_o = ps_pv.tile([P, 512], FP32, tag="pv")
            for t in range(NKT):
                mm = nc.tensor.matmul(
                    psum_o[:],
                    lhsT=pT[:, t, :],
                    rhs=k_pv[:, t],
                    start=(t == 0),
                    stop=(t == NKT - 1),
                )
                if t == 0 and after is not None:
                    # Keep PV(i) behind S(i+1) on the PE queue so PV's work
                    # hides the max->exp latency of tile i+1.
                    tile.add_dep_helper(
                        mm.ins, after.ins, False, "pv-after-next-S"
                    )
            out_sb = work.tile([P, 512], FP32, tag="out_sb")
            nc.vector.tensor_scalar_mul(out_sb[:], psum_o[:], rinv[:])
            nc.sync.dma_start(out_tiles_d[i], out_sb[:])

        for _ in range(repeat_c):
            state = {}
            psum_s, negmax, last_mm = emit_S(0)
            state[0] = (psum_s, negmax, *emit_E(0, psum_s, negmax))
            for i in range(NQT):
                psum_s, negmax, p, rinv = state.pop(i)
                pT = emit_T(i, p)
                after = None
                if i + 1 < NQT:
                    s_ps, s_nm, after = emit_S(i + 1)
                    state[i + 1] = (s_ps, s_nm, *emit_E(i + 1, s_ps, s_nm))
                emit_PV(i, pT, rinv, after=after)


_NC_CACHE = {}


def _get_nc(score_dtype=FP32R, repeat_c=1):
    key = (str(score_dtype), repeat_c)
    if key not in _NC_CACHE:
        _NC_CACHE[key] = build(score_dtype, repeat_c)
    return _NC_CACHE[key]


def kernel(query: np.ndarray, key: np.ndarray) -> np.ndarray:
    query = np.asarray(query, dtype=np.float32)
    key = np.asarray(key, dtype=np.float32)
    assert query.shape == (B, NQ, D) and key.shape == (B, NK, D)
    nc = _get_nc()
    in_maps = [{"query": query[b], "key": key[b]} for b in range(B)]
    res = run_bass_kernel_spmd(nc, in_maps, list(range(B)))
    return np.stack([res.results[b]["out"] for b in range(B)], axis=0)


# revision 14
# speedup vs baseline: 51.8783x; 51.8783x over previous
"""TRN2 Bass kernel for nn_Attention_70257075028315.

reference:
    scores = einsum('bqd,bkd->bqk', query, key)       # B=8, Nq=Nk=2048, D=512
    probs  = softmax(scores, -1)
    out    = einsum('bqk,bkd->bqd', probs, key)

Sharding: batch b -> NeuronCore b (data parallel, fully local attention).

Per-core program (q/k: [2048, 512] fp32):
  Phase A/B: load K and Q in [128, 512] tiles; PE-transpose each into
    kT/qT [128(d), 4(dc), 16(kk/q tile), 128] stored as float32r (rounded by
    the PSUM->SBUF copy); K also cast to float32r natural layout for PV.
  Phase C (per q-tile, software-pipelined):
    S    = qT.T @ kT           accumulated over 4 d-chunks -> PSUM [128, 4, 512]
    max  per 512-chunk on DVE, combined and negated -> bias
    p    = exp(S - max) via one ACT pass PSUM->SBUF (f32r), fused row-sum
    pT   = PE-transpose of p (16x [128,128]) -> PSUM -> ACT copy to SBUF f32r
    o    = pT.T @ k_pv         accumulated over 16 kk-tiles -> PSUM [128, 512]
    out  = o * (1/rowsum)      on DVE, then DMA to DRAM.

float32r matmuls run at 1 cyc/row (vs 4 for fp32) with ~1.5e-2 max abs error
on N(0,512) scores (measured on HW) -> ~1e-3 relative error on the output.
"""

import numpy as np

import concourse.bass as bass
import concourse.tile as tile
import concourse.mybir as mybir
from concourse import bacc
from concourse.bass_utils import run_bass_kernel_spmd
from concourse.masks import make_identity

FP32 = mybir.dt.float32
FP32R = mybir.dt.float32r
AF = mybir.ActivationFunctionType

B, NQ, NK, D = 8, 2048, 2048, 512
P = 128
NKT = NK // P   # 16 kk tiles
NQT = NQ // P   # 16 q tiles
NDC = D // P    # 4 d chunks
NCH = NK // 512  # 4 score chunks of 512


def build(score_dtype=FP32R, repeat_c=1, timed=False):
    """timed=True adds an int32 [1,1] input "reps": phase C re-runs in a
    dynamic For_i loop `reps` more times (0 = just the normal kernel), so one
    NEFF can measure the phase-C slope against itself."""
    nc = bacc.Bacc("TRN2", target_bir_lowering=False, debug=False)
    q_d = nc.dram_tensor("query", [NQ, D], FP32, kind="ExternalInput").ap()
    k_d = nc.dram_tensor("key", [NK, D], FP32, kind="ExternalInput").ap()
    reps_d = None
    if timed:
        reps_d = nc.dram_tensor(
            "reps", [1, 1], mybir.dt.int32, kind="ExternalInput"
        ).ap()
    out_d = nc.dram_tensor("out", [NQ, D], FP32, kind="ExternalOutput").ap()

    q_tiles_d = q_d.rearrange("(t p) d -> t p d", p=P)
    k_tiles_d = k_d.rearrange("(t p) d -> t p d", p=P)
    out_tiles_d = out_d.rearrange("(t p) d -> t p d", p=P)

    with tile.TileContext(nc) as tc:
        _body(tc, q_tiles_d, k_tiles_d, out_tiles_d, score_dtype, repeat_c,
              reps_d)
    nc.compile()
    return nc


def _body(tc, q_tiles_d, k_tiles_d, out_tiles_d, score_dtype, repeat_c,
          reps_d=None):
    from contextlib import ExitStack

    nc = tc.nc
    reps_rv = None
    if reps_d is not None:
        regs = nc.alloc_registers("reps_regs")
        nc.regs_load(regs, reps_d[0:1, 0:1])
        reps_rv = nc.snap(regs, donate=True, min_val=0, max_val=64)
    with ExitStack() as ctx:
        persist = ctx.enter_context(tc.tile_pool(name="persist", bufs=1))
        work = ctx.enter_context(tc.tile_pool(name="work", bufs=2))
        small = ctx.enter_context(tc.tile_pool(name="small", bufs=3))
        ps_s = ctx.enter_context(tc.tile_pool(name="ps_s", bufs=4, space="PSUM"))
        ps_tr = ctx.enter_context(tc.tile_pool(name="ps_tr", bufs=2, space="PSUM"))
        ps_pv = ctx.enter_context(tc.tile_pool(name="ps_pv", bufs=2, space="PSUM"))

        ident = persist.tile([P, P], FP32)
        make_identity(nc, ident[:])
        ident_r = persist.tile([P, P], FP32R)
        nc.vector.tensor_copy(ident_r[:], ident[:])

        # Transposed operands: [d%128, d-chunk, kk-tile, 128]
        kT = persist.tile([P, NDC, NKT, P], score_dtype)
        qT = persist.tile([P, NDC, NQT, P], score_dtype)
        k_pv = persist.tile([P, NKT, 512 // P, P], FP32R)  # natural [kk, d]

        # ---- Phase A/B: load, round to f32r, transpose ----
        # All PSUM->SBUF copies ride the DMA engines (f32r -> f32r, already
        # rounded); compute engines only do the f32 -> f32r rounding casts.
        with tc.tile_pool(name="load", bufs=6) as load:
            for src_d, dstT, pv in ((k_tiles_d, kT, k_pv), (q_tiles_d, qT, None)):
                for g in range(4):  # groups of 4 tiles
                    rtiles = []
                    for j in range(4):
                        t = load.tile([P, D], FP32, tag="ld")
                        nc.sync.dma_start(t[:], src_d[g * 4 + j])
                        if pv is not None:
                            r = pv[:, g * 4 + j]
                            nc.vector.tensor_copy(
                                r, t[:].rearrange("p (a b) -> p a b", b=P)
                            )
                            rtiles.append(pv[:, g * 4 + j].rearrange("p a b -> p (a b)"))
                        else:
                            r = load.tile([P, D], FP32R, tag="ldr")
                            nc.vector.tensor_copy(r[:], t[:])
                            rtiles.append(r[:])
                    for dc in range(NDC):
                        ptr = ps_tr.tile([P, 4, P], FP32R, tag="tr")
                        for j in range(4):
                            nc.tensor.transpose(
                                ptr[:, j, :],
                                rtiles[j][:, dc * P : (dc + 1) * P],
                                ident_r[:],
                            )
                        nc.scalar.copy(dstT[:, dc, g * 4 : (g + 1) * 4, :], ptr[:])

        # ---- Phase C: attention over q tiles, software-pipelined ----
        def emit_S(i):
            """S matmuls (4 separate PSUM chunk tiles) + chunk maxes + negmax."""
            chunks = []
            m4 = small.tile([P, NCH], FP32, tag="m4")
            negmax = small.tile([P, 1], FP32, tag="negmax")
            last_mm = None
            for c in range(NCH):
                psc = ps_s.tile([P, 512], FP32, tag="s")
                for dc in range(NDC):
                    last_mm = nc.tensor.matmul(
                        psc[:],
                        lhsT=qT[:, dc, i, :],
                        rhs=kT[:, dc, c * 4 : (c + 1) * 4, :],
                        start=(dc == 0),
                        stop=(dc == NDC - 1),
                    )
                nc.vector.reduce_max(
                    m4[:, c : c + 1], psc[:], axis=mybir.AxisListType.X
                )
                chunks.append(psc)
            nc.vector.reduce_max(
                negmax[:], m4[:], axis=mybir.AxisListType.X, negate=True
            )
            return chunks, negmax, last_mm

        def emit_E(i, chunks, negmax):
            """exp(S - max) per chunk -> p (f32r) + partial row-sums; 1/sum."""
            p = work.tile([P, NCH, 512], FP32R, tag="p")
            rs4 = small.tile([P, NCH], FP32, tag="rs4")
            rowsum = small.tile([P, 1], FP32, tag="rowsum")
            rinv = small.tile([P, 1], FP32, tag="rinv")
            for c in range(NCH):
                nc.scalar.activation(
                    p[:, c, :], chunks[c][:], AF.Exp, bias=negmax[:],
                    accum_out=rs4[:, c : c + 1],
                )
            nc.vector.reduce_sum(rowsum[:], rs4[:], axis=mybir.AxisListType.X)
            nc.vector.reciprocal(rinv[:], rowsum[:])
            return p, rinv

        def emit_T(i, p):
            """Transpose p -> pT [128(kk), 16 tiles, 128(q)] f32r."""
            pT = work.tile([P, NKT, P], FP32R, tag="pT")
            for g in range(4):
                ptr = ps_tr.tile([P, 4, P], FP32R, tag="tr")
                for j in range(4):
                    nc.tensor.transpose(
                        ptr[:, j, :],
                        p[:, g, j * P : (j + 1) * P],
                        ident_r[:],
                    )
                eng = nc.scalar.copy if g % 2 == 0 else nc.vector.tensor_copy
                eng(pT[:, g * 4 : (g + 1) * 4, :], ptr[:])
            return pT

        def emit_PV(i, pT, rinv, after=None):
            psum_o = ps_pv.tile([P, 512], FP32, tag="pv")
            for t in range(NKT):
                mm = nc.tensor.matmul(
                    psum_o[:],
                    lhsT=pT[:, t, :],
                    rhs=k_pv[:, t],
                    start=(t == 0),
                    stop=(t == NKT - 1),
                )
                if t == 0 and after is not None:
                    # Keep PV(i) behind S(i+1) on the PE queue so PV's work
                    # hides the max->exp latency of tile i+1.
                    tile.add_dep_helper(
                        mm.ins, after.ins, False, "pv-after-next-S"
                    )
            out_sb = work.tile([P, 512], FP32, tag="out_sb")
            nc.vector.tensor_scalar_mul(out_sb[:], psum_o[:], rinv[:])
            nc.sync.dma_start(out_tiles_d[i], out_sb[:])

        def emit_C():
            state = {}
            chunks, negmax, last_mm = emit_S(0)
            state[0] = (chunks, negmax, *emit_E(0, chunks, negmax))
            for i in range(NQT):
                chunks, negmax, p, rinv = state.pop(i)
                pT = emit_T(i, p)
                after = None
                if i + 1 < NQT:
                    s_ps, s_nm, after = emit_S(i + 1)
                    state[i + 1] = (s_ps, s_nm, *emit_E(i + 1, s_ps, s_nm))
                emit_PV(i, pT, rinv, after=after)

        for _ in range(repeat_c):
            emit_C()

        if reps_rv is not None:
            with tc.For_i(0, reps_rv, 1):
                emit_C()


_NC_CACHE = {}


def _get_nc(score_dtype=FP32R, repeat_c=1):
    key = (str(score_dtype), repeat_c)
    if key not in _NC_CACHE:
        _NC_CACHE[key] = build(score_dtype, repeat_c)
    return _NC_CACHE[key]


def kernel(query: np.ndarray, key: np.ndarray) -> np.ndarray:
    query = np.asarray(query, dtype=np.float32)
    key = np.asarray(key, dtype=np.float32)
    assert query.shape == (B, NQ, D) and key.shape == (B, NK, D)
    nc = _get_nc()
    in_maps = [{"query": query[b], "key": key[b]} for b in range(B)]
    res = run_bass_kernel_spmd(nc, in_maps, list(range(B)))
    return np.stack([res.results[b]["out"] for b in range(B)], axis=0)
